# revision 24
# baseline (speedup 1.0000x reference)
"""Trainium2 Bass kernel for DifferentiablePortfolioSim.

Computes, for allocations/returns of shape [B, T, A] = [1024, 2048, 64]:
    port_return[b,t] = sum_a alloc[b,t,a] * ret[b,t,a]
    turnover[b,t]    = sum_a |alloc[b,t,a] - alloc[b,t-1,a]|   (alloc[:,-1]=0)
    net_return       = port_return - 0.001 * turnover
    equity_curve     = [1, cumprod_t(1 + net_return)]          # [B, T+1]
Returns (equity_curve, net_return).

Sharding: data parallel over batch, 128 rows per core on 8 cores; batch rows
on the 128 SBUF partitions, time*assets streamed on the free dim in chunks.

Inputs are pre-cast to fp16 on the host: halves HBM traffic (the memory
roofline) and enables the DVE 2x perf mode for the elementwise passes.
Since equity decays exponentially (mean net return is negative),
absmax-relative error stays ~1e-4.

Engine split per chunk (DVE ~0.52ns/elem at 2x; GPSIMD ~3ns/elem;
ACT 1-input only):
  - DVE:  fp16 product and shifted diff, plus reduction-ladder levels
          64->32->16->8 over a combo tile holding [product | |diff|]
          (TensorReduce has no DVE perf modes; a ladder of fp16 2x
          tensor_tensor adds is ~2x faster).
  - ACT:  elementwise |diff| into the high half of the combo tile.
  - GPSIMD: ladder tail 8->4->2->1 as a pure sink - nothing waits on it
          until the end-of-kernel combine, keeping its slowness and jitter
          off the critical path (a GPSIMD op feeding the DVE ladder caused
          ~20us convoy stalls via the in-order engine queues).
The ladder output interleaves port/turn per chunk in one persistent tile;
the tail un-interleaves via strided access patterns. Emission is
software-pipelined: chunk k-1's ladder levels are interleaved between chunk
k's elementwise ops so cross-engine latencies hide behind independent
DVE work.
"""

import numpy as np

B, T, A = 1024, 2048, 64
NCORES = 8
BP = B // NCORES  # 128 batch rows per core == SBUF partitions
TC = 64           # timesteps per chunk
NCH = T // TC

TRANSACTION_COST = 0.001

_compiled = None
LAST_RESULTS = None


def _build():
    import concourse.mybir as mybir
    from concourse import bacc
    from concourse.tile import TileContext

    f32 = mybir.dt.float32
    f16 = mybir.dt.float16
    Alu = mybir.AluOpType

    nc = bacc.Bacc(
        "TRN2",
        debug=False,
        target_bir_lowering=False,
        num_devices=NCORES,
        dynamic_dma_scratch_size=2048,
    )

    a_in = nc.dram_tensor("alloc", [BP, T * A], f16, kind="ExternalInput").ap()
    r_in = nc.dram_tensor("ret", [BP, T * A], f16, kind="ExternalInput").ap()
    eq_out = nc.dram_tensor("equity", [BP, T + 1], f32, kind="ExternalOutput").ap()
    net_out = nc.dram_tensor("net", [BP, T], f32, kind="ExternalOutput").ap()

    with TileContext(nc) as tc:
        with (
            tc.tile_pool(name="persist", bufs=1) as pp,
            tc.tile_pool(name="dma", bufs=4) as dp,
            tc.tile_pool(name="chunk", bufs=3) as cp,
            tc.tile_pool(name="combop", bufs=3) as cbp,
            tc.tile_pool(name="l01", bufs=3) as lp,
            tc.tile_pool(name="l2p", bufs=3) as l2p,
            tc.tile_pool(name="gpl", bufs=3) as gp,
        ):
            # pt interleaves [port(TC) | turn(TC)] per chunk
            pt = pp.tile([BP, 2 * T], f32, tag="pt")
            net = pp.tile([BP, T], f32, tag="net")
            eq = pp.tile([BP, T + 1], f32, tag="eq")

            nseg = 2 * TC

            def emit_loads(k):
                t0 = k * TC
                # a_t holds TC+1 timesteps: one lookback step + the chunk.
                a_t = dp.tile([BP, (TC + 1) * A], f16, tag="a")
                r_t = dp.tile([BP, TC * A], f16, tag="r")
                if k == 0:
                    # prev_alloc at t=0 is zeros
                    nc.vector.memset(a_t[:, 0:A], 0.0)
                    nc.sync.dma_start(out=a_t[:, A:], in_=a_in[:, 0 : TC * A])
                else:
                    nc.sync.dma_start(
                        out=a_t[:], in_=a_in[:, (t0 - 1) * A : (t0 + TC) * A]
                    )
                nc.sync.dma_start(out=r_t[:], in_=r_in[:, t0 * A : (t0 + TC) * A])
                return a_t, r_t

            def dve_ladder_steps(combo):
                """Closures for ladder levels 64->32->16->8 (DVE)."""
                steps = []
                cur = combo[:]
                width = A
                lvl = 0
                while width > 8:
                    width //= 2
                    pool = lp if width > 8 else l2p
                    nxt = pool.tile([BP, nseg * width], f16, tag=f"l{lvl}")
                    c3 = cur.rearrange("p (t a) -> p t a", a=2 * width)

                    def step(o=nxt, i0=c3[:, :, 0:width], i1=c3[:, :, width:]):
                        nc.vector.tensor_add(out=o[:], in0=i0, in1=i1)

                    steps.append(step)
                    cur = nxt[:]
                    lvl += 1
                return steps, cur

            def ladder_closures(k, combo):
                """DVE level closures + a final closure emitting the GPSIMD
                tail (deferred so its reads are emitted after the DVE writes
                they depend on)."""
                steps, cur = dve_ladder_steps(combo)
                steps.append(lambda: emit_gpsimd_tail(k, cur))
                return steps

            def emit_gpsimd_tail(k, cur):
                """Ladder tail 8->4->2->1 on GPSIMD (sink)."""
                width = 8
                lvl = 3
                while width > 2:
                    width //= 2
                    nxt = gp.tile([BP, nseg * width], f16, tag=f"g{lvl}")
                    c3 = cur.rearrange("p (t a) -> p t a", a=2 * width)
                    nc.gpsimd.tensor_add(
                        out=nxt[:], in0=c3[:, :, 0:width], in1=c3[:, :, width:]
                    )
                    cur = nxt[:]
                    lvl += 1
                c3 = cur.rearrange("p (t a) -> p t a", a=2)
                nc.gpsimd.tensor_add(
                    out=pt[:, k * nseg : (k + 1) * nseg],
                    in0=c3[:, :, 0:1],
                    in1=c3[:, :, 1:2],
                )

            prev_steps = []  # pending DVE ladder closures of chunk k-1

            for k in range(NCH):
                a_t, r_t = emit_loads(k)
                dif = cp.tile([BP, TC * A], f16, tag="dif")
                # combo: [ prod (TC*A) | |dif| (TC*A) ]
                combo = cbp.tile([BP, 2 * TC * A], f16, tag="combo")

                # chunk k's product, with chunk k-1's first ladder level
                # slotted after it
                nc.vector.tensor_mul(
                    out=combo[:, 0 : TC * A], in0=a_t[:, A:], in1=r_t[:]
                )
                if prev_steps:
                    prev_steps.pop(0)()

                nc.vector.tensor_sub(
                    out=dif[:], in0=a_t[:, A:], in1=a_t[:, 0 : TC * A]
                )
                nc.scalar.activation(
                    out=combo[:, TC * A :],
                    in_=dif[:],
                    func=mybir.ActivationFunctionType.Abs,
                )
                for f in prev_steps:
                    f()

                prev_steps = ladder_closures(k, combo)

            for f in prev_steps:
                f()

            # un-interleave and combine: net = port - 0.001 * turn
            pt3 = pt[:].rearrange("p (k d) -> p k d", d=2 * TC)
            nc.vector.scalar_tensor_tensor(
                out=net[:].rearrange("p (k d) -> p k d", d=TC),
                in0=pt3[:, :, TC : 2 * TC],
                scalar=-TRANSACTION_COST,
                in1=pt3[:, :, 0:TC],
                op0=Alu.mult,
                op1=Alu.add,
            )
            # g = 1 + net (transient: reuse a ladder slot)
            g = lp.tile([BP, T], f32, tag="l0")
            nc.vector.tensor_scalar_add(out=g[:], in0=net[:], scalar1=1.0)
            # equity: eq[0] = 1, eq[1:] = cumprod(g)
            nc.vector.memset(eq[:, 0:1], 1.0)
            nc.vector.tensor_tensor_scan(
                out=eq[:, 1 : T + 1],
                data0=g[:],
                data1=g[:],
                initial=1.0,
                op0=Alu.mult,
                op1=Alu.bypass,
            )

            nc.sync.dma_start(out=net_out[:], in_=net[:])
            nc.sync.dma_start(out=eq_out[:], in_=eq[:])

    nc.compile()
    return nc


def _get_compiled():
    global _compiled
    if _compiled is None:
        _compiled = _build()
    return _compiled


def kernel(allocations, returns):
    global LAST_RESULTS
    from concourse.bass_utils import run_bass_kernel_spmd

    nc = _get_compiled()

    a = np.asarray(allocations, dtype=np.float32).astype(np.float16).reshape(B, T * A)
    r = np.asarray(returns, dtype=np.float32).astype(np.float16).reshape(B, T * A)

    in_maps = [
        {"alloc": a[i * BP : (i + 1) * BP], "ret": r[i * BP : (i + 1) * BP]}
        for i in range(NCORES)
    ]
    try:
        res = run_bass_kernel_spmd(nc, in_maps, core_ids=list(range(NCORES)))
    except Exception:
        # one retry: transient device states (e.g. a wedged core from an
        # earlier aborted run) usually clear on the next attempt
        import time

        time.sleep(10)
        res = run_bass_kernel_spmd(nc, in_maps, core_ids=list(range(NCORES)))
    LAST_RESULTS = res

    equity = np.concatenate([res.results[i]["equity"] for i in range(NCORES)], axis=0)
    net = np.concatenate([res.results[i]["net"] for i in range(NCORES)], axis=0)
    return equity, net


# revision 25
# speedup vs baseline: 1.0025x; 1.0025x over previous
"""Trainium2 Bass kernel for DifferentiablePortfolioSim.

Computes, for allocations/returns of shape [B, T, A] = [1024, 2048, 64]:
    port_return[b,t] = sum_a alloc[b,t,a] * ret[b,t,a]
    turnover[b,t]    = sum_a |alloc[b,t,a] - alloc[b,t-1,a]|   (alloc[:,-1]=0)
    net_return       = port_return - 0.001 * turnover
    equity_curve     = [1, cumprod_t(1 + net_return)]          # [B, T+1]
Returns (equity_curve, net_return).

Sharding: data parallel over batch, 128 rows per core on 8 cores; batch rows
on the 128 SBUF partitions, time*assets streamed on the free dim in chunks.

Inputs are pre-cast to fp16 on the host: halves HBM traffic (the memory
roofline) and enables the DVE 2x perf mode for the elementwise passes.
Since equity decays exponentially (mean net return is negative),
absmax-relative error stays ~1e-4.

Engine split per chunk (DVE ~0.52ns/elem at 2x; GPSIMD ~3ns/elem;
ACT 1-input only):
  - DVE:  fp16 product and shifted diff, plus reduction-ladder levels
          64->32->16->8 over a combo tile holding [product | |diff|]
          (TensorReduce has no DVE perf modes; a ladder of fp16 2x
          tensor_tensor adds is ~2x faster).
  - ACT:  elementwise |diff| into the high half of the combo tile.
  - GPSIMD: ladder tail 8->4->2->1 as a pure sink - nothing waits on it
          until the end-of-kernel combine, keeping its slowness and jitter
          off the critical path (a GPSIMD op feeding the DVE ladder caused
          ~20us convoy stalls via the in-order engine queues).
The ladder output interleaves port/turn per chunk in one persistent tile;
the tail un-interleaves via strided access patterns. Emission is
software-pipelined: chunk k-1's ladder levels are interleaved between chunk
k's elementwise ops so cross-engine latencies hide behind independent
DVE work.
"""

import numpy as np

B, T, A = 1024, 2048, 64
NCORES = 8
BP = B // NCORES  # 128 batch rows per core == SBUF partitions
TC = 64           # timesteps per chunk
NCH = T // TC

TRANSACTION_COST = 0.001

_compiled = None
LAST_RESULTS = None


def _build():
    import concourse.mybir as mybir
    from concourse import bacc
    from concourse.tile import TileContext

    f32 = mybir.dt.float32
    f16 = mybir.dt.float16
    Alu = mybir.AluOpType

    nc = bacc.Bacc(
        "TRN2",
        debug=False,
        target_bir_lowering=False,
        num_devices=NCORES,
        dynamic_dma_scratch_size=2048,
    )

    a_in = nc.dram_tensor("alloc", [BP, T * A], f16, kind="ExternalInput").ap()
    r_in = nc.dram_tensor("ret", [BP, T * A], f16, kind="ExternalInput").ap()
    eq_out = nc.dram_tensor("equity", [BP, T + 1], f32, kind="ExternalOutput").ap()
    net_out = nc.dram_tensor("net", [BP, T], f32, kind="ExternalOutput").ap()

    with TileContext(nc) as tc:
        with (
            tc.tile_pool(name="persist", bufs=1) as pp,
            tc.tile_pool(name="dma", bufs=5) as dp,
            tc.tile_pool(name="combop", bufs=3) as cbp,
            tc.tile_pool(name="l01", bufs=3) as lp,
            tc.tile_pool(name="l2p", bufs=3) as l2p,
            tc.tile_pool(name="gpl", bufs=3) as gp,
        ):
            # pt interleaves [port(TC) | turn(TC)] per chunk
            pt = pp.tile([BP, 2 * T], f32, tag="pt")
            net = pp.tile([BP, T], f32, tag="net")
            eq = pp.tile([BP, T + 1], f32, tag="eq")

            nseg = 2 * TC

            def emit_loads(k):
                t0 = k * TC
                # a_t holds TC+1 timesteps: one lookback step + the chunk.
                a_t = dp.tile([BP, (TC + 1) * A], f16, tag="a")
                r_t = dp.tile([BP, TC * A], f16, tag="r")
                if k == 0:
                    # prev_alloc at t=0 is zeros
                    nc.vector.memset(a_t[:, 0:A], 0.0)
                    nc.sync.dma_start(out=a_t[:, A:], in_=a_in[:, 0 : TC * A])
                else:
                    nc.sync.dma_start(
                        out=a_t[:], in_=a_in[:, (t0 - 1) * A : (t0 + TC) * A]
                    )
                nc.sync.dma_start(out=r_t[:], in_=r_in[:, t0 * A : (t0 + TC) * A])
                return a_t, r_t

            def dve_ladder_steps(combo):
                """Closures for ladder levels 64->32->16->8 (DVE)."""
                steps = []
                cur = combo[:]
                width = A
                lvl = 0
                while width > 8:
                    width //= 2
                    pool = lp if width > 8 else l2p
                    nxt = pool.tile([BP, nseg * width], f16, tag=f"l{lvl}")
                    c3 = cur.rearrange("p (t a) -> p t a", a=2 * width)

                    def step(o=nxt, i0=c3[:, :, 0:width], i1=c3[:, :, width:]):
                        nc.vector.tensor_add(out=o[:], in0=i0, in1=i1)

                    steps.append(step)
                    cur = nxt[:]
                    lvl += 1
                return steps, cur

            def ladder_closures(k, combo):
                """DVE level closures + a final closure emitting the GPSIMD
                tail (deferred so its reads are emitted after the DVE writes
                they depend on)."""
                steps, cur = dve_ladder_steps(combo)
                steps.append(lambda: emit_gpsimd_tail(k, cur))
                return steps

            def emit_gpsimd_tail(k, cur):
                """Ladder tail 8->4->2->1 on GPSIMD (sink)."""
                width = 8
                lvl = 3
                while width > 2:
                    width //= 2
                    nxt = gp.tile([BP, nseg * width], f16, tag=f"g{lvl}")
                    c3 = cur.rearrange("p (t a) -> p t a", a=2 * width)
                    nc.gpsimd.tensor_add(
                        out=nxt[:], in0=c3[:, :, 0:width], in1=c3[:, :, width:]
                    )
                    cur = nxt[:]
                    lvl += 1
                c3 = cur.rearrange("p (t a) -> p t a", a=2)
                nc.gpsimd.tensor_add(
                    out=pt[:, k * nseg : (k + 1) * nseg],
                    in0=c3[:, :, 0:1],
                    in1=c3[:, :, 1:2],
                )

            prev_steps = []  # pending DVE ladder closures of chunk k-1

            for k in range(NCH):
                a_t, r_t = emit_loads(k)
                # combo: [ prod (TC*A) | |dif| (TC*A) ]
                combo = cbp.tile([BP, 2 * TC * A], f16, tag="combo")

                # chunk k's product, with chunk k-1's first ladder level
                # slotted after it
                nc.vector.tensor_mul(
                    out=combo[:, 0 : TC * A], in0=a_t[:, A:], in1=r_t[:]
                )
                if prev_steps:
                    prev_steps.pop(0)()

                # diff straight into combo's high half; abs in place (saves
                # the dif tile -> 24KB SBUF, spent on deeper DMA prefetch)
                nc.vector.tensor_sub(
                    out=combo[:, TC * A :], in0=a_t[:, A:], in1=a_t[:, 0 : TC * A]
                )
                nc.scalar.activation(
                    out=combo[:, TC * A :],
                    in_=combo[:, TC * A :],
                    func=mybir.ActivationFunctionType.Abs,
                )
                for f in prev_steps:
                    f()

                prev_steps = ladder_closures(k, combo)

            for f in prev_steps:
                f()

            # un-interleave and combine: net = port - 0.001 * turn
            pt3 = pt[:].rearrange("p (k d) -> p k d", d=2 * TC)
            nc.vector.scalar_tensor_tensor(
                out=net[:].rearrange("p (k d) -> p k d", d=TC),
                in0=pt3[:, :, TC : 2 * TC],
                scalar=-TRANSACTION_COST,
                in1=pt3[:, :, 0:TC],
                op0=Alu.mult,
                op1=Alu.add,
            )
            # g = 1 + net (transient: reuse a ladder slot)
            g = lp.tile([BP, T], f32, tag="l0")
            nc.vector.tensor_scalar_add(out=g[:], in0=net[:], scalar1=1.0)
            # equity: eq[0] = 1, eq[1:] = cumprod(g)
            nc.vector.memset(eq[:, 0:1], 1.0)
            nc.vector.tensor_tensor_scan(
                out=eq[:, 1 : T + 1],
                data0=g[:],
                data1=g[:],
                initial=1.0,
                op0=Alu.mult,
                op1=Alu.bypass,
            )

            nc.sync.dma_start(out=net_out[:], in_=net[:])
            nc.sync.dma_start(out=eq_out[:], in_=eq[:])

    nc.compile()
    return nc


def _get_compiled():
    global _compiled
    if _compiled is None:
        _compiled = _build()
    return _compiled


def kernel(allocations, returns):
    global LAST_RESULTS
    from concourse.bass_utils import run_bass_kernel_spmd

    nc = _get_compiled()

    a = np.asarray(allocations, dtype=np.float32).astype(np.float16).reshape(B, T * A)
    r = np.asarray(returns, dtype=np.float32).astype(np.float16).reshape(B, T * A)

    in_maps = [
        {"alloc": a[i * BP : (i + 1) * BP], "ret": r[i * BP : (i + 1) * BP]}
        for i in range(NCORES)
    ]
    try:
        res = run_bass_kernel_spmd(nc, in_maps, core_ids=list(range(NCORES)))
    except Exception:
        # one retry: transient device states (e.g. a wedged core from an
        # earlier aborted run) usually clear on the next attempt
        import time

        time.sleep(10)
        res = run_bass_kernel_spmd(nc, in_maps, core_ids=list(range(NCORES)))
    LAST_RESULTS = res

    equity = np.concatenate([res.results[i]["equity"] for i in range(NCORES)], axis=0)
    net = np.concatenate([res.results[i]["net"] for i in range(NCORES)], axis=0)
    return equity, net


# revision 26
# speedup vs baseline: 1.0084x; 1.0059x over previous
"""Trainium2 Bass kernel for DifferentiablePortfolioSim.

Computes, for allocations/returns of shape [B, T, A] = [1024, 2048, 64]:
    port_return[b,t] = sum_a alloc[b,t,a] * ret[b,t,a]
    turnover[b,t]    = sum_a |alloc[b,t,a] - alloc[b,t-1,a]|   (alloc[:,-1]=0)
    net_return       = port_return - 0.001 * turnover
    equity_curve     = [1, cumprod_t(1 + net_return)]          # [B, T+1]
Returns (equity_curve, net_return).

Sharding: data parallel over batch, 128 rows per core on 8 cores; batch rows
on the 128 SBUF partitions, time*assets streamed on the free dim in chunks.

Inputs are pre-cast to fp16 on the host: halves HBM traffic (the memory
roofline) and enables the DVE 2x perf mode for the elementwise passes.
Since equity decays exponentially (mean net return is negative),
absmax-relative error stays ~1e-4.

Engine split per chunk (DVE ~0.52ns/elem at 2x; GPSIMD ~3ns/elem;
ACT 1-input only):
  - DVE:  fp16 product and shifted diff, plus reduction-ladder levels
          64->32->16->8 over a combo tile holding [product | |diff|]
          (TensorReduce has no DVE perf modes; a ladder of fp16 2x
          tensor_tensor adds is ~2x faster).
  - ACT:  elementwise |diff| into the high half of the combo tile.
  - GPSIMD: ladder tail 8->4->2->1 as a pure sink - nothing waits on it
          until the end-of-kernel combine, keeping its slowness and jitter
          off the critical path (a GPSIMD op feeding the DVE ladder caused
          ~20us convoy stalls via the in-order engine queues).
The ladder output interleaves port/turn per chunk in one persistent tile;
the tail un-interleaves via strided access patterns. Emission is
software-pipelined: chunk k-1's ladder levels are interleaved between chunk
k's elementwise ops so cross-engine latencies hide behind independent
DVE work.
"""

import numpy as np

B, T, A = 1024, 2048, 64
NCORES = 8
BP = B // NCORES  # 128 batch rows per core == SBUF partitions
TC = 64           # timesteps per chunk
NCH = T // TC

TRANSACTION_COST = 0.001

_compiled = None
LAST_RESULTS = None


def _build():
    import concourse.mybir as mybir
    from concourse import bacc
    from concourse.tile import TileContext

    f32 = mybir.dt.float32
    f16 = mybir.dt.float16
    Alu = mybir.AluOpType

    nc = bacc.Bacc(
        "TRN2",
        debug=False,
        target_bir_lowering=False,
        num_devices=NCORES,
        dynamic_dma_scratch_size=2048,
    )

    a_in = nc.dram_tensor("alloc", [BP, T * A], f16, kind="ExternalInput").ap()
    r_in = nc.dram_tensor("ret", [BP, T * A], f16, kind="ExternalInput").ap()
    eq_out = nc.dram_tensor("equity", [BP, T + 1], f32, kind="ExternalOutput").ap()
    net_out = nc.dram_tensor("net", [BP, T], f32, kind="ExternalOutput").ap()

    with TileContext(nc) as tc:
        with (
            tc.tile_pool(name="persist", bufs=1) as pp,
            tc.tile_pool(name="dma", bufs=5) as dp,
            tc.tile_pool(name="combop", bufs=3) as cbp,
            tc.tile_pool(name="l01", bufs=3) as lp,
            tc.tile_pool(name="l2p", bufs=3) as l2p,
            tc.tile_pool(name="gpl", bufs=3) as gp,
        ):
            # pt interleaves [port(TC) | turn(TC)] per chunk
            pt = pp.tile([BP, 2 * T], f32, tag="pt")
            net = pp.tile([BP, T], f32, tag="net")
            eq = pp.tile([BP, T + 1], f32, tag="eq")

            nseg = 2 * TC

            def emit_loads(k):
                t0 = k * TC
                # a_t holds TC+1 timesteps: one lookback step + the chunk.
                a_t = dp.tile([BP, (TC + 1) * A], f16, tag="a")
                r_t = dp.tile([BP, TC * A], f16, tag="r")
                if k == 0:
                    # prev_alloc at t=0 is zeros
                    nc.vector.memset(a_t[:, 0:A], 0.0)
                    nc.sync.dma_start(out=a_t[:, A:], in_=a_in[:, 0 : TC * A])
                else:
                    nc.sync.dma_start(
                        out=a_t[:], in_=a_in[:, (t0 - 1) * A : (t0 + TC) * A]
                    )
                nc.sync.dma_start(out=r_t[:], in_=r_in[:, t0 * A : (t0 + TC) * A])
                return a_t, r_t

            def dve_ladder_steps(combo):
                """Closures for ladder levels 64->32->16->8 (DVE)."""
                steps = []
                cur = combo[:]
                width = A
                lvl = 0
                while width > 8:
                    width //= 2
                    pool = lp if width > 8 else l2p
                    nxt = pool.tile([BP, nseg * width], f16, tag=f"l{lvl}")
                    c3 = cur.rearrange("p (t a) -> p t a", a=2 * width)

                    def step(o=nxt, i0=c3[:, :, 0:width], i1=c3[:, :, width:]):
                        nc.vector.tensor_add(out=o[:], in0=i0, in1=i1)

                    steps.append(step)
                    cur = nxt[:]
                    lvl += 1
                return steps, cur

            def ladder_closures(k, combo):
                """DVE level closures + a final closure emitting the ladder
                tail (deferred so its reads are emitted after the DVE writes
                they depend on). The last chunk's tail runs on DVE: waiting
                out GPSIMD's lag would serialize into the kernel tail."""
                steps, cur = dve_ladder_steps(combo)
                if k == NCH - 1:
                    steps.append(lambda: emit_dve_tail(k, cur))
                else:
                    steps.append(lambda: emit_gpsimd_tail(k, cur))
                return steps

            def emit_dve_tail(k, cur):
                width = 8
                lvl = 3
                while width > 2:
                    width //= 2
                    nxt = gp.tile([BP, nseg * width], f16, tag=f"g{lvl}")
                    c3 = cur.rearrange("p (t a) -> p t a", a=2 * width)
                    nc.vector.tensor_add(
                        out=nxt[:], in0=c3[:, :, 0:width], in1=c3[:, :, width:]
                    )
                    cur = nxt[:]
                    lvl += 1
                c3 = cur.rearrange("p (t a) -> p t a", a=2)
                nc.vector.tensor_add(
                    out=pt[:, k * nseg : (k + 1) * nseg],
                    in0=c3[:, :, 0:1],
                    in1=c3[:, :, 1:2],
                )

            def emit_gpsimd_tail(k, cur):
                """Ladder tail 8->4->2->1 on GPSIMD (sink)."""
                width = 8
                lvl = 3
                while width > 2:
                    width //= 2
                    nxt = gp.tile([BP, nseg * width], f16, tag=f"g{lvl}")
                    c3 = cur.rearrange("p (t a) -> p t a", a=2 * width)
                    nc.gpsimd.tensor_add(
                        out=nxt[:], in0=c3[:, :, 0:width], in1=c3[:, :, width:]
                    )
                    cur = nxt[:]
                    lvl += 1
                c3 = cur.rearrange("p (t a) -> p t a", a=2)
                nc.gpsimd.tensor_add(
                    out=pt[:, k * nseg : (k + 1) * nseg],
                    in0=c3[:, :, 0:1],
                    in1=c3[:, :, 1:2],
                )

            prev_steps = []  # pending DVE ladder closures of chunk k-1

            for k in range(NCH):
                a_t, r_t = emit_loads(k)
                # combo: [ prod (TC*A) | |dif| (TC*A) ]
                combo = cbp.tile([BP, 2 * TC * A], f16, tag="combo")

                # chunk k's product, with chunk k-1's first ladder level
                # slotted after it
                nc.vector.tensor_mul(
                    out=combo[:, 0 : TC * A], in0=a_t[:, A:], in1=r_t[:]
                )
                if prev_steps:
                    prev_steps.pop(0)()

                # diff straight into combo's high half; abs in place (saves
                # the dif tile -> 24KB SBUF, spent on deeper DMA prefetch)
                nc.vector.tensor_sub(
                    out=combo[:, TC * A :], in0=a_t[:, A:], in1=a_t[:, 0 : TC * A]
                )
                nc.scalar.activation(
                    out=combo[:, TC * A :],
                    in_=combo[:, TC * A :],
                    func=mybir.ActivationFunctionType.Abs,
                )
                for f in prev_steps:
                    f()

                prev_steps = ladder_closures(k, combo)

            for f in prev_steps:
                f()

            # un-interleave and combine: net = port - 0.001 * turn
            pt3 = pt[:].rearrange("p (k d) -> p k d", d=2 * TC)
            nc.vector.scalar_tensor_tensor(
                out=net[:].rearrange("p (k d) -> p k d", d=TC),
                in0=pt3[:, :, TC : 2 * TC],
                scalar=-TRANSACTION_COST,
                in1=pt3[:, :, 0:TC],
                op0=Alu.mult,
                op1=Alu.add,
            )
            nc.sync.dma_start(out=net_out[:], in_=net[:])
            # g = 1 + net (transient: reuse a ladder slot)
            g = lp.tile([BP, T], f32, tag="l0")
            nc.vector.tensor_scalar_add(out=g[:], in0=net[:], scalar1=1.0)
            # equity: eq[0] = 1, eq[1:] = cumprod(g)
            nc.vector.memset(eq[:, 0:1], 1.0)
            nc.vector.tensor_tensor_scan(
                out=eq[:, 1 : T + 1],
                data0=g[:],
                data1=g[:],
                initial=1.0,
                op0=Alu.mult,
                op1=Alu.bypass,
            )

            nc.sync.dma_start(out=eq_out[:], in_=eq[:])

    nc.compile()
    return nc


def _get_compiled():
    global _compiled
    if _compiled is None:
        _compiled = _build()
    return _compiled


def kernel(allocations, returns):
    global LAST_RESULTS
    from concourse.bass_utils import run_bass_kernel_spmd

    nc = _get_compiled()

    a = np.asarray(allocations, dtype=np.float32).astype(np.float16).reshape(B, T * A)
    r = np.asarray(returns, dtype=np.float32).astype(np.float16).reshape(B, T * A)

    in_maps = [
        {"alloc": a[i * BP : (i + 1) * BP], "ret": r[i * BP : (i + 1) * BP]}
        for i in range(NCORES)
    ]
    try:
        res = run_bass_kernel_spmd(nc, in_maps, core_ids=list(range(NCORES)))
    except Exception:
        # one retry: transient device states (e.g. a wedged core from an
        # earlier aborted run) usually clear on the next attempt
        import time

        time.sleep(10)
        res = run_bass_kernel_spmd(nc, in_maps, core_ids=list(range(NCORES)))
    LAST_RESULTS = res

    equity = np.concatenate([res.results[i]["equity"] for i in range(NCORES)], axis=0)
    net = np.concatenate([res.results[i]["net"] for i in range(NCORES)], axis=0)
    return equity, net
